# revision 4
# baseline (speedup 1.0000x reference)
"""ConvergedInhibition TRN2 kernel, v5 (fp8 DoubleRow).

The reference computes, per pixel (n,h,w), an FFT deconvolution along the
channel axis: y = ifft(fft(x)/fft(k)).real. Since k is fixed, this is a
circular convolution with g = ifft(1/fft(k)): a dense CxC circulant matmul
per pixel. g is a near-delta: one big tap a0=1.055 at offset 288 plus a
small tail h (||h||/||g|| = 0.13). Split y = a0*shift288(x) + h (*) x:
the device computes only the correction z = H @ x with H the circulant of
h (scaled by S=16), in fp8 end-to-end; the host adds the exact identity
part from the fp32 input it already holds (same class of host work as the
previous version's un-rotate gather). Error budget: ~5.7e-3 total (fp8
quantization of x, H and z; no truncation - full K=512) vs the 2e-2 gate.

fp8 cuts HBM traffic to 12.85 MB/core (~36us at 358 GB/s/core) and, with
perf_mode=DoubleRow (2 fp8 weights per PE cell, K=256 per matmul), the
PE work to 2 matmuls per [128,392] psum tile (~47us). Both well under the
89us fp16 baseline.

Layout: per core 4 images, A_n = [C=512, HW=3136] fp8. a_sb holds the 4
K-chunks side by side; a DoubleRow matmul pairs chunks (g, g+2) via a
3-D AP [p, 2, m] with dim-1 stride 2*HW (rhs) / 2*C (weights), so no
interleaved data layout is needed.
"""

import numpy as np
import ml_dtypes

import concourse.bass as bass  # noqa: F401  (registers bass types)
import concourse.mybir as mybir
from concourse import bacc
from concourse.bass_utils import run_bass_kernel_spmd

N_CORES = 8
N, C, H, W = 32, 512, 56, 56
HW = H * W                      # 3136
IMGS = N // N_CORES             # 4 images per core
P = 128                         # partitions
NCHUNK = C // P                 # 4
PT = 392                        # pixel tile (free dim), 3136 = 8*392
NPT = HW // PT                  # 8
CB = 784                        # pixel column block, 3136 = 4*784
NCB = HW // CB                  # 4
P2 = NPT // NCB                 # 2 pixel tiles per column block
ROT = 288                       # position of g's dominant (identity) tap
SCALE = 16.0                    # folded into H so z uses e4m3's sweet spot
IO_DT = mybir.dt.float8e4
IO_NP = ml_dtypes.float8_e4m3   # == mybir.dt.np(float8e4)

_CACHE = {}

TILES_PER_CB = NCHUNK * P2      # 8 psum tiles per (img, cb)
TILES_PER_IMG = NCB * TILES_PER_CB  # 32


def _tidx(img, cb, zc, p2):
    return img * TILES_PER_IMG + cb * TILES_PER_CB + zc * P2 + p2


def _build_nc():
    """Hand-rolled engine programs with explicit semaphores.

    Streams:
      Sync:   act loads, one per (img, cb): [128, 4 chunks x 784B] = 401KB
      Scalar: gt load, casts for zc in {2,3}, all output stores
      Tensor: per psum tile, 2 DoubleRow matmuls (K=256 each -> K=512)
      Vector: casts for zc in {0,1}
    psum bank = zc*2 + p2 (same producer/consumer pairing every (img,cb)
    group); a_sb double-buffered over images, gated by s_mm.
    """
    nc = bacc.Bacc("TRN2", target_bir_lowering=False, debug=False,
                   num_devices=N_CORES)
    act = nc.dram_tensor("act", [IMGS, C, HW], IO_DT, kind="ExternalInput")
    gt = nc.dram_tensor("gt", [C, C], IO_DT, kind="ExternalInput")
    out = nc.dram_tensor("out", [IMGS, C, HW], IO_DT, kind="ExternalOutput")

    # src AP for one (img, cb) load: [p, jc, m] matching a_sb dest dims
    act_v = act.ap().rearrange("n (jc p) m -> n p jc m", p=P)
    gt_v = gt.ap().rearrange("(jc p) r -> p jc r", p=P)
    out_v = out.ap().rearrange("n (zc p) m -> n zc p m", p=P)

    from contextlib import ExitStack
    with ExitStack() as ctx:
        a_sb = [ctx.enter_context(
            nc.sbuf_tensor(f"a_sb{h}", [P, NCHUNK * HW], IO_DT)).ap()
            for h in range(2)]
        gt_sb = ctx.enter_context(
            nc.sbuf_tensor("gt_sb", [P, NCHUNK * C], IO_DT)).ap()
        o_sb = [[ctx.enter_context(
            nc.sbuf_tensor(f"o_sb{i}_{z}", [P, HW], IO_DT)).ap()
            for z in range(NCHUNK)] for i in range(IMGS)]
        psum = [ctx.enter_context(
            nc.psum_tensor(f"ps{i}", [P, 512], mybir.dt.float32)).ap()
            for i in range(8)]

        # DoubleRow pair views: dim layout [p, two, g, .] with chunk
        # jc = two*2 + g, so slicing g gives the (g, g+2) pair.
        a_pair = [a.rearrange("p (two g m) -> p two g m", two=2, g=2)
                  for a in a_sb]
        gt_pair = gt_sb.rearrange("p (two g c) -> p two g c", two=2, g=2)

        s_gt = nc.alloc_semaphore("s_gt")
        s_ld = [[nc.alloc_semaphore(f"s_ld{h}_{cb}") for cb in range(NCB)]
                for h in range(2)]
        s_mm = nc.alloc_semaphore("s_mm")
        s_cast_v = nc.alloc_semaphore("s_cast_v")
        s_cast_s = nc.alloc_semaphore("s_cast_s")
        s_st = nc.alloc_semaphore("s_st")
        all_sems = ([s_gt, s_mm, s_cast_v, s_cast_s, s_st]
                    + [s for row in s_ld for s in row])

        # Stage 0: clear semaphores (not zeroed on alloc; must not carry
        # values across executions). Block-exit barrier orders this.
        with nc.Block("clears") as blk:

            @blk.sync
            def _(sync):
                for s in all_sems:
                    sync.sem_clear(s)

        with nc.Block("main") as blk:

            def emit_load(sync, img, cb):
                if img >= 2:
                    # a_sb[img%2] cols for cb must be fully consumed
                    sync.wait_ge(s_mm, _tidx(img - 2, cb, NCHUNK - 1, P2 - 1) + 1)
                sync.dma_start(
                    a_sb[img % 2].rearrange("p (jc m) -> p jc m", jc=NCHUNK)[
                        :, :, cb * CB:(cb + 1) * CB],
                    act_v[img, :, :, cb * CB:(cb + 1) * CB],
                ).then_inc(s_ld[img % 2][cb], 16)

            @blk.sync
            def _(sync):
                for img in range(min(2, IMGS)):
                    for cb in range(NCB):
                        emit_load(sync, img, cb)
                for img in range(2, IMGS):
                    for cb in range(NCB):
                        emit_load(sync, img, cb)

            @blk.scalar
            def _(scalar):
                scalar.dma_start(
                    gt_sb.rearrange("p (jc c) -> p jc c", jc=NCHUNK), gt_v,
                ).then_inc(s_gt, 16)
                n_store = 0
                for img in range(IMGS):
                    for cb in range(NCB):
                        for zc in (2, 3):
                            for p2 in range(P2):
                                t = _tidx(img, cb, zc, p2)
                                p = cb * P2 + p2
                                scalar.wait_ge(s_mm, t + 1)
                                scalar.copy(
                                    o_sb[img][zc][:, p * PT:(p + 1) * PT],
                                    psum[zc * 2 + p2][:, :PT],
                                ).then_inc(s_cast_s)
                    # all casts of this img done (zc2,3 by program order
                    # above; zc0,1 once vector's count reaches 16*(img+1))
                    scalar.wait_ge(s_cast_v, 16 * (img + 1))
                    for zc in range(NCHUNK):
                        scalar.dma_start(
                            out_v[img, zc], o_sb[img][zc],
                        ).then_inc(s_st, 16)
                        n_store += 1
                scalar.wait_ge(s_st, 16 * n_store)

            @blk.tensor
            def _(tensor):
                tensor.wait_ge(s_gt, 16)
                # HAM warmup while the first act loads land: dummy matmuls
                # on gt data into bank 7 (overwritten by the first real
                # group before its first read; start=True resets)
                for _ in range(12):
                    tensor.matmul(psum[7][:, :PT], gt_sb[:, :P],
                                  gt_sb[:, :PT], start=True, stop=True)
                for img in range(IMGS):
                    for cb in range(NCB):
                        tensor.wait_ge(s_ld[img % 2][cb],
                                       16 * (img // 2 + 1))
                        gidx = img * NCB + cb
                        for zc in range(NCHUNK):
                            for p2 in range(P2):
                                bank = zc * 2 + p2
                                if gidx >= 1:
                                    # previous cast of this bank done
                                    sem = s_cast_v if zc < 2 else s_cast_s
                                    cnt = (gidx - 1) * 4 + (zc % 2) * 2 + p2 + 1
                                    tensor.wait_ge(sem, cnt)
                                p = cb * P2 + p2
                                for g in range(2):
                                    mm = tensor.matmul(
                                        psum[bank][:, :PT],
                                        gt_pair[:, :, g,
                                                zc * P:(zc + 1) * P],
                                        a_pair[img % 2][:, :, g,
                                                        p * PT:(p + 1) * PT],
                                        start=(g == 0), stop=(g == 1),
                                        perf_mode=mybir.MatmulPerfMode.DoubleRow,
                                    )
                                mm.then_inc(s_mm)

            @blk.vector
            def _(vector):
                for img in range(IMGS):
                    for cb in range(NCB):
                        for zc in (0, 1):
                            for p2 in range(P2):
                                t = _tidx(img, cb, zc, p2)
                                p = cb * P2 + p2
                                vector.wait_ge(s_mm, t + 1)
                                vector.tensor_copy(
                                    o_sb[img][zc][:, p * PT:(p + 1) * PT],
                                    psum[zc * 2 + p2][:, :PT],
                                ).then_inc(s_cast_v)

    nc.compile()
    return nc


def _make_g(inhib_kernel: np.ndarray) -> np.ndarray:
    k = np.asarray(inhib_kernel, dtype=np.float64)
    return np.real(np.fft.ifft(1.0 / np.fft.fft(k)))


def _make_gt(inhib_kernel: np.ndarray) -> np.ndarray:
    """Weights HT[j, i] = S * h[(i-j) mod C] in fp8, h = g minus its
    dominant tap a0 at offset ROT (added back exactly on the host)."""
    g = _make_g(inhib_kernel)
    h = g.copy()
    h[ROT] -= g[ROT]
    idx = (np.arange(C)[None, :] - np.arange(C)[:, None]) % C
    return np.ascontiguousarray((SCALE * h[idx]).astype(IO_NP))


def kernel(activations, inhib_kernel):
    acts = np.asarray(activations, dtype=np.float32)
    assert acts.shape == (N, C, H, W), acts.shape
    g = _make_g(np.asarray(inhib_kernel))
    a0 = g[ROT]
    gt_np = _make_gt(np.asarray(inhib_kernel))

    if "nc" not in _CACHE:
        _CACHE["nc"] = _build_nc()
    nc = _CACHE["nc"]

    acts_flat = acts.reshape(N, C, HW)
    acts8 = acts_flat.astype(IO_NP)
    in_maps = [
        {"act": np.ascontiguousarray(acts8[c * IMGS:(c + 1) * IMGS]),
         "gt": gt_np}
        for c in range(N_CORES)
    ]
    res = run_bass_kernel_spmd(nc, in_maps, core_ids=list(range(N_CORES)))
    z = np.concatenate([np.asarray(r["out"]) for r in res.results], axis=0)
    # y = a0 * x[(i-ROT) mod C] + z[i]/S, in fp32 on the host
    y = z.astype(np.float32)
    y *= np.float32(1.0 / SCALE)
    y += np.float32(a0) * np.roll(acts_flat, ROT, axis=1)
    return y.reshape(N, C, H, W)


# revision 5
# speedup vs baseline: 1.0490x; 1.0490x over previous
"""ConvergedInhibition TRN2 kernel, v5 (fp8 DoubleRow).

The reference computes, per pixel (n,h,w), an FFT deconvolution along the
channel axis: y = ifft(fft(x)/fft(k)).real. Since k is fixed, this is a
circular convolution with g = ifft(1/fft(k)): a dense CxC circulant matmul
per pixel. g is a near-delta: one big tap a0=1.055 at offset 288 plus a
small tail h (||h||/||g|| = 0.13). Split y = a0*shift288(x) + h (*) x:
the device computes only the correction z = H @ x with H the circulant of
h (scaled by S=16), in fp8 end-to-end; the host adds the exact identity
part from the fp32 input it already holds (same class of host work as the
previous version's un-rotate gather). Error budget: ~5.7e-3 total (fp8
quantization of x, H and z; no truncation - full K=512) vs the 2e-2 gate.

fp8 cuts HBM traffic to 12.85 MB/core (~36us at 358 GB/s/core) and, with
perf_mode=DoubleRow (2 fp8 weights per PE cell, K=256 per matmul), the
PE work to 2 matmuls per [128,392] psum tile (~47us). Both well under the
89us fp16 baseline.

Layout: per core 4 images, A_n = [C=512, HW=3136] fp8. a_sb holds the 4
K-chunks side by side; a DoubleRow matmul pairs chunks (g, g+2) via a
3-D AP [p, 2, m] with dim-1 stride 2*HW (rhs) / 2*C (weights), so no
interleaved data layout is needed.
"""

import numpy as np
import ml_dtypes

import concourse.bass as bass  # noqa: F401  (registers bass types)
import concourse.mybir as mybir
from concourse import bacc
from concourse.bass_utils import run_bass_kernel_spmd

N_CORES = 8
N, C, H, W = 32, 512, 56, 56
HW = H * W                      # 3136
IMGS = N // N_CORES             # 4 images per core
P = 128                         # partitions
NCHUNK = C // P                 # 4
PT = 392                        # pixel tile (free dim), 3136 = 8*392
NPT = HW // PT                  # 8
CB = 784                        # pixel column block, 3136 = 4*784
NCB = HW // CB                  # 4
P2 = NPT // NCB                 # 2 pixel tiles per column block
ROT = 288                       # position of g's dominant (identity) tap
SCALE = 16.0                    # folded into H so z uses e4m3's sweet spot
IO_DT = mybir.dt.float8e4
IO_NP = ml_dtypes.float8_e4m3   # == mybir.dt.np(float8e4)

_CACHE = {}

TILES_PER_CB = NCHUNK * P2      # 8 psum tiles per (img, cb)
TILES_PER_IMG = NCB * TILES_PER_CB  # 32


def _tidx(img, cb, zc, p2):
    return img * TILES_PER_IMG + cb * TILES_PER_CB + zc * P2 + p2


def _build_nc():
    """Hand-rolled engine programs with explicit semaphores.

    Streams:
      Sync:   act loads, one per (img, cb): [128, 4 chunks x 784B] = 401KB
      Scalar: gt load, casts for zc in {2,3}, all output stores
      Tensor: per psum tile, 2 DoubleRow matmuls (K=256 each -> K=512)
      Vector: casts for zc in {0,1}
    psum bank = zc*2 + p2 (same producer/consumer pairing every (img,cb)
    group); a_sb double-buffered over images, gated by s_mm.
    """
    nc = bacc.Bacc("TRN2", target_bir_lowering=False, debug=False,
                   num_devices=N_CORES)
    act = nc.dram_tensor("act", [IMGS, C, HW], IO_DT, kind="ExternalInput")
    gt = nc.dram_tensor("gt", [C, C], IO_DT, kind="ExternalInput")
    out = nc.dram_tensor("out", [IMGS, C, HW], IO_DT, kind="ExternalOutput")

    # src AP for one (img, cb) load: [p, jc, m] matching a_sb dest dims
    act_v = act.ap().rearrange("n (jc p) m -> n p jc m", p=P)
    gt_v = gt.ap().rearrange("(jc p) r -> p jc r", p=P)
    out_v = out.ap().rearrange("n (zc p) m -> n zc p m", p=P)

    from contextlib import ExitStack
    with ExitStack() as ctx:
        a_sb = [ctx.enter_context(
            nc.sbuf_tensor(f"a_sb{h}", [P, NCHUNK * HW], IO_DT)).ap()
            for h in range(2)]
        gt_sb = ctx.enter_context(
            nc.sbuf_tensor("gt_sb", [P, NCHUNK * C], IO_DT)).ap()
        o_sb = [[ctx.enter_context(
            nc.sbuf_tensor(f"o_sb{i}_{z}", [P, HW], IO_DT)).ap()
            for z in range(NCHUNK)] for i in range(IMGS)]
        psum = [ctx.enter_context(
            nc.psum_tensor(f"ps{i}", [P, 512], mybir.dt.float32)).ap()
            for i in range(8)]

        # DoubleRow pair views: dim layout [p, two, g, .] with chunk
        # jc = two*2 + g, so slicing g gives the (g, g+2) pair.
        a_pair = [a.rearrange("p (two g m) -> p two g m", two=2, g=2)
                  for a in a_sb]
        gt_pair = gt_sb.rearrange("p (two g c) -> p two g c", two=2, g=2)

        s_gt = nc.alloc_semaphore("s_gt")
        s_ld = [[nc.alloc_semaphore(f"s_ld{h}_{cb}") for cb in range(NCB)]
                for h in range(2)]
        s_mm = nc.alloc_semaphore("s_mm")
        s_cast_v = nc.alloc_semaphore("s_cast_v")
        s_cast_s = nc.alloc_semaphore("s_cast_s")
        s_st = nc.alloc_semaphore("s_st")
        all_sems = ([s_gt, s_mm, s_cast_v, s_cast_s, s_st]
                    + [s for row in s_ld for s in row])

        # Stage 0: clear semaphores (not zeroed on alloc; must not carry
        # values across executions). Block-exit barrier orders this.
        with nc.Block("clears") as blk:

            @blk.sync
            def _(sync):
                for s in all_sems:
                    sync.sem_clear(s)

        with nc.Block("main") as blk:

            def emit_load(sync, img, cb):
                if img >= 2:
                    # a_sb[img%2] cols for cb must be fully consumed
                    sync.wait_ge(s_mm, _tidx(img - 2, cb, NCHUNK - 1, P2 - 1) + 1)
                sync.dma_start(
                    a_sb[img % 2].rearrange("p (jc m) -> p jc m", jc=NCHUNK)[
                        :, :, cb * CB:(cb + 1) * CB],
                    act_v[img, :, :, cb * CB:(cb + 1) * CB],
                ).then_inc(s_ld[img % 2][cb], 16)

            @blk.sync
            def _(sync):
                # gt first: everything waits on it, so it must not queue
                # behind the 3.2MB act prefetch on the shared SDMA engines
                sync.dma_start(
                    gt_sb.rearrange("p (jc c) -> p jc c", jc=NCHUNK), gt_v,
                ).then_inc(s_gt, 16)
                for img in range(min(2, IMGS)):
                    for cb in range(NCB):
                        emit_load(sync, img, cb)
                for img in range(2, IMGS):
                    for cb in range(NCB):
                        emit_load(sync, img, cb)

            @blk.scalar
            def _(scalar):
                n_store = 0
                for img in range(IMGS):
                    for cb in range(NCB):
                        for zc in (2, 3):
                            for p2 in range(P2):
                                t = _tidx(img, cb, zc, p2)
                                p = cb * P2 + p2
                                scalar.wait_ge(s_mm, t + 1)
                                scalar.copy(
                                    o_sb[img][zc][:, p * PT:(p + 1) * PT],
                                    psum[zc * 2 + p2][:, :PT],
                                ).then_inc(s_cast_s)
                        # store this (img, cb) quarter of every zc as soon
                        # as its casts land (spreads the drain; short tail)
                        scalar.wait_ge(s_cast_v, img * 16 + cb * 4 + 4)
                        for zc in range(NCHUNK):
                            scalar.dma_start(
                                out_v[img, zc, :, cb * CB:(cb + 1) * CB],
                                o_sb[img][zc][:, cb * CB:(cb + 1) * CB],
                            ).then_inc(s_st, 16)
                            n_store += 1
                scalar.wait_ge(s_st, 16 * n_store)

            @blk.tensor
            def _(tensor):
                tensor.wait_ge(s_gt, 16)
                # HAM warmup while the first act loads land: dummy matmuls
                # on gt data into bank 7 (overwritten by the first real
                # group before its first read; start=True resets)
                for _ in range(12):
                    tensor.matmul(psum[7][:, :PT], gt_sb[:, :P],
                                  gt_sb[:, :PT], start=True, stop=True)
                for img in range(IMGS):
                    for cb in range(NCB):
                        tensor.wait_ge(s_ld[img % 2][cb],
                                       16 * (img // 2 + 1))
                        gidx = img * NCB + cb
                        for zc in range(NCHUNK):
                            for p2 in range(P2):
                                bank = zc * 2 + p2
                                if gidx >= 1:
                                    # previous cast of this bank done
                                    sem = s_cast_v if zc < 2 else s_cast_s
                                    cnt = (gidx - 1) * 4 + (zc % 2) * 2 + p2 + 1
                                    tensor.wait_ge(sem, cnt)
                                p = cb * P2 + p2
                                for g in range(2):
                                    mm = tensor.matmul(
                                        psum[bank][:, :PT],
                                        gt_pair[:, :, g,
                                                zc * P:(zc + 1) * P],
                                        a_pair[img % 2][:, :, g,
                                                        p * PT:(p + 1) * PT],
                                        start=(g == 0), stop=(g == 1),
                                        perf_mode=mybir.MatmulPerfMode.DoubleRow,
                                    )
                                mm.then_inc(s_mm)

            @blk.vector
            def _(vector):
                for img in range(IMGS):
                    for cb in range(NCB):
                        for zc in (0, 1):
                            for p2 in range(P2):
                                t = _tidx(img, cb, zc, p2)
                                p = cb * P2 + p2
                                vector.wait_ge(s_mm, t + 1)
                                vector.tensor_copy(
                                    o_sb[img][zc][:, p * PT:(p + 1) * PT],
                                    psum[zc * 2 + p2][:, :PT],
                                ).then_inc(s_cast_v)

    nc.compile()
    return nc


def _make_g(inhib_kernel: np.ndarray) -> np.ndarray:
    k = np.asarray(inhib_kernel, dtype=np.float64)
    return np.real(np.fft.ifft(1.0 / np.fft.fft(k)))


def _make_gt(inhib_kernel: np.ndarray) -> np.ndarray:
    """Weights HT[j, i] = S * h[(i-j) mod C] in fp8, h = g minus its
    dominant tap a0 at offset ROT (added back exactly on the host)."""
    g = _make_g(inhib_kernel)
    h = g.copy()
    h[ROT] -= g[ROT]
    idx = (np.arange(C)[None, :] - np.arange(C)[:, None]) % C
    return np.ascontiguousarray((SCALE * h[idx]).astype(IO_NP))


def kernel(activations, inhib_kernel):
    acts = np.asarray(activations, dtype=np.float32)
    assert acts.shape == (N, C, H, W), acts.shape
    g = _make_g(np.asarray(inhib_kernel))
    a0 = g[ROT]
    gt_np = _make_gt(np.asarray(inhib_kernel))

    if "nc" not in _CACHE:
        _CACHE["nc"] = _build_nc()
    nc = _CACHE["nc"]

    acts_flat = acts.reshape(N, C, HW)
    acts8 = acts_flat.astype(IO_NP)
    in_maps = [
        {"act": np.ascontiguousarray(acts8[c * IMGS:(c + 1) * IMGS]),
         "gt": gt_np}
        for c in range(N_CORES)
    ]
    res = run_bass_kernel_spmd(nc, in_maps, core_ids=list(range(N_CORES)))
    z = np.concatenate([np.asarray(r["out"]) for r in res.results], axis=0)
    # y = a0 * x[(i-ROT) mod C] + z[i]/S, in fp32 on the host
    y = z.astype(np.float32)
    y *= np.float32(1.0 / SCALE)
    y += np.float32(a0) * np.roll(acts_flat, ROT, axis=1)
    return y.reshape(N, C, H, W)
